# revision 9
# baseline (speedup 1.0000x reference)
"""GroupedQueryAttention TRN2 Bass kernel, 8-way (batch x head-group) parallel.

B=2, S=2048, E=2048, H=16 q-heads, KVH=4 kv-heads, HD=128.
Core d handles batch d//4 and head group g=d%4: q-heads {4g..4g+3} plus
kv-head g.  No replicated projection work: per-core PE load is the perfect
1/8 split of total MACs (vs. head-only sharding which recomputes K/V and
the full-token Q projection on every core).  Each core emits a partial
out[S, E] (summed over its 4 heads' E-slice of out_proj); the host sums
the 4 partials per batch -- device time only pays a 16MB write per core.

Layout strategy (everything transposed so all matmuls use natural layouts):
  phase 1: qT/kT/vT[hd, tok] = W.T @ xT  (lhsT = W chunks, rhs = xT chunks)
           + RoPE applied on PSUM->SBUF epilogue (1/sqrt(HD) folded into wq)
           Tile 0 accumulates all 6 outputs chunk-major (smallest possible
           first-DMA footprint); tiles 1..3 run two 3-output passes
           (gA=q0,q1,k then gB=q2,q3,v) so pass-A drains overlap pass-B
           matmuls and PSUM banks never stall the PE.  v chunks are
           PE-transposed to natural layout right after each tile's drain.
  attention (per head, 512-wide q tile): flash-style over PAIRS of 128-wide
           kt chunks: scoresT[kt, qt] = kT_chunk.T @ qT_tile -> one exp over
           both chunks [128,1024] (no max subtraction; scores are ~N(0,1))
           -> causal mask on diagonal chunks -> ctxT[hd, qt] += v_chunk.T @ P;
           sumexp[1, qt] += ones.T @ P.  PE runs one chunk pair ahead of ACT
           so exp latency is hidden.  Normalize ctxT with broadcast sums
           (all-ones stationary) + reciprocal_approx_fast.
  out_proj (per q tile, right after that tile's attention so its output DMA
           overlaps the next q tile's attention): out[tok, e] = sum_h
           ctxT_h_chunk.T @ wo_h (partial over this core's 512 head dims;
           host sums the 4 partials per batch).

All matmul operands are float32r (full PE rate at N>=512; ~1e-3 precision).
"""
import sys
sys.path.insert(0, '/opt/trn_rl_repo')

import numpy as np
from contextlib import ExitStack

import concourse.bass as bass
import concourse.bacc as bacc
import concourse.tile as tile
from concourse import mybir
from concourse.bass_utils import run_bass_kernel_spmd
from concourse.alu_op_type import AluOpType

F32 = mybir.dt.float32
F32R = mybir.dt.float32r
EXP = mybir.ActivationFunctionType.Exp

B, S, E = 2, 2048, 2048
H, KVH, HD = 16, 4, 128
NH = H // KVH              # 4 q-heads per core (one kv head)
T = B * S
NCORES = 8
NT = 512                   # token tile (matmul free dim)
NTT = S // NT              # 4 token tiles per core
KC = E // 128              # 16 contraction chunks for projections
KB = S // 128              # 16 kt chunks per core batch
ROPE_BASE = 10000.0

_CACHE = {}


def _emit(nc, tc, ctx):
    xT_d = nc.declare_dram_parameter("xT", [E, S], F32R, isOutput=False)
    wq_d = nc.declare_dram_parameter("wq", [E, NH * HD], F32R, isOutput=False)
    wk_d = nc.declare_dram_parameter("wk", [E, HD], F32R, isOutput=False)
    wv_d = nc.declare_dram_parameter("wv", [E, HD], F32R, isOutput=False)
    wo_d = nc.declare_dram_parameter("wo", [NH * HD, E], F32R, isOutput=False)
    cos_d = nc.declare_dram_parameter("cos", [HD, S], F32, isOutput=False)
    sinm_d = nc.declare_dram_parameter("sinm", [HD, S], F32, isOutput=False)
    masks_d = nc.declare_dram_parameter("masks", [4, 128, NT], F32, isOutput=False)
    ident_d = nc.declare_dram_parameter("ident", [128, 128], F32R, isOutput=False)
    onec_d = nc.declare_dram_parameter("onec", [128, 128], F32R, isOutput=False)
    out_d = nc.declare_dram_parameter("out", [S, E], F32, isOutput=True)

    persist = ctx.enter_context(tc.tile_pool(name="persist", bufs=1))
    qTs = persist.tile([HD, NH, S], F32R)       # 4 q heads, transposed
    kT = persist.tile([HD, S], F32R)
    v_sb = persist.tile([128, KB, HD], F32R)    # v natural: [tok%128, blk, hd]
    ident = persist.tile([128, 128], F32R)
    ones_col = persist.tile([128, 128], F32R)
    scr = persist.tile([1, 128], F32)
    nc.sync.dma_start(ident[:], ident_d[:, :])
    nc.sync.dma_start(ones_col[:], onec_d[:, :])
    # preload the exp ACT table set (~2.7us) while the PE is on projections
    nc.scalar.activation(scr[:], ident[0:1, :], EXP)

    # phase-2 constants live alongside phase-1 pools (loaded late in phase 1)
    mpool = ctx.enter_context(tc.tile_pool(name="mpool", bufs=1))
    wopool = ctx.enter_context(tc.tile_pool(name="wopool", bufs=1))
    masks_s = mpool.tile([128, 4, NT], F32)
    wo_sb = wopool.tile([HD, NH, E], F32R)

    # ---------------- phase 1: projections + RoPE ----------------
    with ExitStack() as p1:
        wpool = p1.enter_context(tc.tile_pool(name="wpool", bufs=1))
        trig = p1.enter_context(tc.tile_pool(name="trig", bufs=2))
        xpool = p1.enter_context(tc.tile_pool(name="xpool", bufs=1))
        rope = p1.enter_context(tc.tile_pool(name="rope", bufs=2))
        vTp = p1.enter_context(tc.tile_pool(name="vTp", bufs=2))
        ps1 = p1.enter_context(tc.tile_pool(name="ps1", bufs=1, space="PSUM"))
        pst = p1.enter_context(tc.tile_pool(name="pst", bufs=2, space="PSUM"))

        wq_s = wpool.tile([128, KC, NH * HD], F32R)
        wk_s = wpool.tile([128, KC, HD], F32R)
        wv_s = wpool.tile([128, KC, HD], F32R)
        wqv = wq_d.rearrange("(k p) m -> p k m", p=128)
        wkv = wk_d.rearrange("(k p) m -> p k m", p=128)
        wvv = wv_d.rearrange("(k p) t -> p k t", p=128)
        xT_view = xT_d.rearrange("(k p) t -> p k t", p=128)

        # spread DMAs over the two HW DGE queues (sync + scalar); a single
        # queue saturates around ~250 GB/s, well below HBM bandwidth
        def load_wchunk(kq):
            ks = slice(4 * kq, 4 * kq + 4)
            nc.scalar.dma_start(wq_s[:, ks, :], wqv[:, ks, :])
            nc.sync.dma_start(wk_s[:, ks, :], wkv[:, ks, :])
            nc.sync.dma_start(wv_s[:, ks, :], wvv[:, ks, :])

        def load_xk(tt):
            t0 = tt * NT
            xk = []
            for kq in range(4):  # 4 DMAs x 4 chunks of [128, NT]
                xt = xpool.tile([128, 4, NT], F32R, tag=f"xk{kq}")
                (nc.sync if kq % 2 == 0 else nc.scalar).dma_start(
                    xt[:], xT_view[:, 4 * kq:4 * kq + 4, t0:t0 + NT])
                xk.append(xt)
            return xk

        def load_trig(tt):
            t0 = tt * NT
            cos_c = trig.tile([HD, NT], F32, tag="cos")
            sinm_c = trig.tile([HD, NT], F32, tag="sinm")
            nc.scalar.dma_start(cos_c[:], cos_d[:, t0:t0 + NT])
            nc.scalar.dma_start(sinm_c[:], sinm_d[:, t0:t0 + NT])
            return cos_c, sinm_c

        # issue order = queue order: tile-0 consumes chunk-major, so
        # interleave w-chunk groups with tile-0 x quarters on both queues --
        # the PE's first matmul needs only ~2.5MB and starts ~6us in, with
        # arrivals roughly pacing consumption after that.
        xk0 = []
        for kq in range(4):
            ks = slice(4 * kq, 4 * kq + 4)
            qa, qb = ((nc.sync, nc.scalar) if kq % 2 == 0
                      else (nc.scalar, nc.sync))
            qa.dma_start(wq_s[:, ks, :], wqv[:, ks, :])
            xt = xpool.tile([128, 4, NT], F32R, tag=f"xk{kq}")
            qb.dma_start(xt[:], xT_view[:, ks, 0:NT])
            xk0.append(xt)
            qb.dma_start(wk_s[:, ks, :], wkv[:, ks, :])
            qa.dma_start(wv_s[:, ks, :], wvv[:, ks, :])
        tcur = load_trig(0)
        xk_next = load_xk(1)
        tnext = load_trig(1)

        def rope_drain(psum, dest, cos_c, sinm_c, cpeng):
            # quick PSUM->SBUF release copy (so the bank frees in ~1us and
            # the next tile's matmuls aren't gated on the full RoPE chain),
            # then q' = q*cos + swap_halves(q)*sinm from SBUF at leisure
            cpy = rope.tile([HD, NT], F32, tag="cpy", bufs=3)
            if cpeng == 0:
                nc.vector.tensor_copy(cpy[:], psum[:])
            else:
                nc.scalar.copy(cpy[:], psum[:])
            sw = rope.tile([HD, NT], F32, tag="sw")
            nc.scalar.copy(sw[0:64, :], cpy[64:128, :])
            nc.scalar.copy(sw[64:128, :], cpy[0:64, :])
            nc.vector.tensor_tensor(sw[:], sw[:], sinm_c[:], AluOpType.mult)
            nc.vector.tensor_tensor(dest, cpy[:], cos_c[:], AluOpType.mult)
            nc.vector.tensor_tensor(dest, dest, sw[:], AluOpType.add)

        def v_transpose(vT_t, tt):
            # vT chunk -> natural v in v_sb via PE transpose
            for i in range(NT // 128):
                tp = pst.tile([128, 128], F32R, tag="tp")
                nc.tensor.matmul(tp[:], vT_t[:, i * 128:(i + 1) * 128],
                                 ident[:], is_transpose=True)
                nc.vector.tensor_copy(v_sb[:, tt * (NT // 128) + i, :], tp[:])

        for tt in range(NTT):
            t0 = tt * NT
            xk = xk0 if tt == 0 else xk_next
            cos_c, sinm_c = tcur
            if 0 < tt < NTT - 1:
                xk_next = load_xk(tt + 1)
                tnext = load_trig(tt + 1)
            if tt == NTT - 1:
                # prefetch phase-2 constants behind the last x loads
                nc.scalar.dma_start(
                    wo_sb[:], wo_d.rearrange("(h p) e -> p h e", p=128))
                nc.sync.dma_start(masks_s[:],
                                  masks_d.rearrange("m p j -> p m j"))

            gA = ps1.tile([HD, 3, NT], F32, tag="gA")
            gB = ps1.tile([HD, 3, NT], F32, tag="gB")
            vT_t = vTp.tile([HD, NT], F32R, tag="vT")

            def wslotsA(k):
                return [(gA, 0, wq_s[:, k, 0:HD]),
                        (gA, 1, wq_s[:, k, HD:2 * HD]),
                        (gA, 2, wk_s[:, k, :])]

            def wslotsB(k):
                return [(gB, 0, wq_s[:, k, 2 * HD:3 * HD]),
                        (gB, 1, wq_s[:, k, 3 * HD:4 * HD]),
                        (gB, 2, wv_s[:, k, :])]

            def mm_pass(slot_fn, drains=()):
                for k in range(KC):
                    xck = xk[k // 4][:, k % 4, :]
                    for (ps, idx, w) in slot_fn(k):
                        nc.tensor.matmul(ps[:, idx, :], w, xck,
                                         start=(k == 0), stop=(k == KC - 1))
                for psum, dest, cpeng in drains:
                    rope_drain(psum, dest, cos_c, sinm_c, cpeng)

            drainsA = [(gA[:, 0, :], qTs[:, 0, t0:t0 + NT], 0),
                       (gA[:, 1, :], qTs[:, 1, t0:t0 + NT], 1),
                       (gA[:, 2, :], kT[:, t0:t0 + NT], 0)]
            if tt == 0:
                # chunk-major: all 6 outputs per chunk -> first matmul only
                # needs w-chunk 0 + the first x quarter (~2.5MB of DMA)
                mm_pass(lambda k: wslotsA(k) + wslotsB(k))
                for psum, dest, cpeng in drainsA:
                    rope_drain(psum, dest, cos_c, sinm_c, cpeng)
            else:
                # two staggered 3-output passes: pass-A drains overlap
                # pass-B matmuls, so PSUM never backpressures the PE
                mm_pass(wslotsA, drainsA)
                mm_pass(wslotsB)
            # vT drains first so the transposes (the next PE work) aren't
            # queued behind the q2/q3 RoPE chains on the scalar engine
            nc.scalar.copy(vT_t[:], gB[:, 2, :])
            v_transpose(vT_t, tt)
            rope_drain(gB[:, 0, :], qTs[:, 2, t0:t0 + NT], cos_c, sinm_c, 0)
            rope_drain(gB[:, 1, :], qTs[:, 3, t0:t0 + NT], cos_c, sinm_c, 0)
            tcur = tnext

    # ---------- attention + out_proj, interleaved per q-tile ----------
    with ExitStack() as p2:
        ppool = p2.enter_context(tc.tile_pool(name="ppool", bufs=4))
        npool = p2.enter_context(tc.tile_pool(name="npool", bufs=2))
        cpool = p2.enter_context(tc.tile_pool(name="cpool", bufs=2))
        obpool = p2.enter_context(tc.tile_pool(name="obpool", bufs=2))
        # PSUM budget (8 banks): scores pairs 2x2 | ctx 1 | sumexp 1 |
        # out_proj 2
        pss = p2.enter_context(tc.tile_pool(name="pss", bufs=2, space="PSUM"))
        psc = p2.enter_context(tc.tile_pool(name="psc", bufs=1, space="PSUM"))
        psn = p2.enter_context(tc.tile_pool(name="psn", bufs=1, space="PSUM"))
        pso = p2.enter_context(tc.tile_pool(name="pso", bufs=2, space="PSUM"))

        def emit_outproj_chunk(ctx_p, qt_p, tc4):
            # one 128-token chunk of out_proj for a finished q tile; called
            # between attention heads so these always-ready matmuls fill the
            # PE bubbles of the exp-gated attention pipeline
            tch = qt_p * (NT // 128) + tc4
            tsl = slice(tch * 128, (tch + 1) * 128)
            csl = slice(tc4 * 128, (tc4 + 1) * 128)
            ob = obpool.tile([128, E], F32, tag="ob", bufs=3)
            for ech in range(E // NT):
                esl = slice(ech * NT, (ech + 1) * NT)
                op = pso.tile([128, NT], F32, tag="o")
                for h in range(NH):
                    nc.tensor.matmul(op[:], ctx_p[:, h, csl],
                                     wo_sb[:, h, esl],
                                     start=(h == 0), stop=(h == NH - 1))
                nc.vector.tensor_copy(ob[:, esl], op[:])
            (nc.sync if tch % 2 == 0 else nc.scalar).dma_start(
                out_d[tsl, :], ob[:])

        pending = []
        for qt in range(NTT):
            npairs = 2 * (qt + 1)
            nk = 2 * npairs
            q_sl = slice(qt * NT, (qt + 1) * NT)
            ctx_t = cpool.tile([HD, NH, NT], F32R, tag="ctx")
            # ---- attention for all 4 heads on this q tile ----
            for h in range(NH):
                if pending:
                    emit_outproj_chunk(*pending.pop(0))
                ctxp = psc.tile([HD, NT], F32, tag="ctx")
                sump = psn.tile([128, NT], F32, tag="sum")

                def delta(kc):
                    # fully-masked column prefix of a diagonal chunk
                    return (kc - 4 * qt) * 128 if kc >= 4 * qt else 0

                def emit_ctx_sum(pexp, padd, j):
                    for half in (0, 1):
                        kc = 2 * j + half
                        dl = delta(kc)
                        st, sp_ = (kc == 0), (kc == nk - 1)
                        nc.tensor.matmul(
                            ctxp[:, dl:], v_sb[:, kc, :],
                            pexp[:, half, dl:], start=st, stop=sp_)
                        if padd is None:  # diagonal pair: per-half sums
                            nc.tensor.matmul(
                                sump[:, dl:], ones_col[:],
                                pexp[:, half, dl:], start=st, stop=sp_)
                    if padd is not None:  # one sum matmul per pair
                        st, sp_ = (j == 0), (j == npairs - 1)
                        nc.tensor.matmul(sump[:], ones_col[:], padd[:],
                                         start=st, stop=sp_)

                prev = None
                for j in range(npairs):
                    diag = j >= 2 * qt
                    sp2 = pss.tile([128, 2, NT], F32, tag="s")
                    for half in (0, 1):
                        kc = 2 * j + half
                        dl = delta(kc)
                        nc.tensor.matmul(
                            sp2[:, half, dl:],
                            kT[:, kc * 128:(kc + 1) * 128],
                            qTs[:, h, q_sl][:, dl:])
                    pexp = ppool.tile([128, 2, NT], F32R, tag="p")
                    padd = None
                    if diag:
                        # per-half exp on valid columns, then causal mask
                        for half in (0, 1):
                            kc = 2 * j + half
                            dl = delta(kc)
                            nc.scalar.activation(pexp[:, half, dl:],
                                                 sp2[:, half, dl:], EXP)
                            nc.vector.tensor_tensor(
                                pexp[:, half, dl:], pexp[:, half, dl:],
                                masks_s[:, kc - 4 * qt, dl:],
                                AluOpType.mult)
                    else:
                        nc.scalar.activation(pexp[:], sp2[:], EXP)
                        padd = ppool.tile([128, NT], F32R, tag="padd")
                        (nc.vector if j % 2 == 0 else nc.gpsimd).tensor_tensor(
                            padd[:], pexp[:, 0, :], pexp[:, 1, :],
                            AluOpType.add)
                    if prev is not None:
                        emit_ctx_sum(*prev)
                    prev = (pexp, padd, j)
                emit_ctx_sum(*prev)
                # drain PSUM into SBUF immediately so the banks free up;
                # sump already holds sumexp broadcast to all partitions
                # (all-ones stationary), so normalize is just recip+mult
                ctmp = npool.tile([HD, NT], F32, tag="ctmp")
                nc.vector.tensor_copy(ctmp[:], ctxp[:])
                bcs = npool.tile([128, NT], F32, tag="bcs")
                nc.vector.reciprocal_approx_fast(bcs[:], sump[:])
                nc.vector.tensor_tensor(ctx_t[:, h, :], ctmp[:], bcs[:],
                                        AluOpType.mult)

            pending = [(ctx_t, qt, tc4) for tc4 in range(NT // 128)]
        for args in pending:
            emit_outproj_chunk(*args)


def _build():
    if "nc" in _CACHE:
        return _CACHE["nc"]
    nc = bacc.Bacc("TRN2", target_bir_lowering=False, debug=False,
                   num_devices=NCORES)
    with tile.TileContext(nc) as tc:
        with nc.allow_low_precision(reason="float32r operands for full-rate PE"):
            with ExitStack() as ctx:
                _emit(nc, tc, ctx)
    nc.compile()
    _CACHE["nc"] = nc
    return nc


def _host_consts():
    if "consts" in _CACHE:
        return _CACHE["consts"]
    # RoPE tables, computed in float32 like the reference
    inv_freq = (1.0 / (ROPE_BASE ** (np.arange(0, HD, 2, dtype=np.float32) / HD))
                ).astype(np.float32)
    t = np.arange(S, dtype=np.float32)
    freqs = np.outer(t, inv_freq).astype(np.float32)          # [S, 64]
    emb = np.concatenate([freqs, freqs], axis=-1)             # [S, HD]
    cos_t = np.ascontiguousarray(np.cos(emb).T.astype(np.float32))  # [HD, S]
    sin_t = np.sin(emb).T.astype(np.float32)
    sinm_t = np.ascontiguousarray(
        np.concatenate([-sin_t[:64], sin_t[64:]], axis=0))
    # causal masks for the 4 diagonal 128-chunk offsets within a 512 q-tile
    p = np.arange(128)[:, None]
    j = np.arange(NT)[None, :]
    masks = np.stack([(m * 128 + p <= j) for m in range(4)]).astype(np.float32)
    ident = np.eye(128, dtype=np.float32)
    _CACHE["consts"] = (cos_t, sinm_t, masks, ident)
    return _CACHE["consts"]


def _in_maps(x, wq, wk, wv, wo):
    cos_t, sinm_t, masks, ident = _host_consts()
    x = np.asarray(x, dtype=np.float32)
    scale = np.float32(1.0 / np.sqrt(HD))
    in_maps = []
    for d in range(NCORES):
        b, g = d // 4, d % 4
        xT = np.ascontiguousarray(x[b].T)                     # [E, S]
        in_maps.append({
            "xT": xT,
            "wq": np.ascontiguousarray(wq[:, g * NH * HD:(g + 1) * NH * HD]) * scale,
            "wk": np.ascontiguousarray(wk[:, g * HD:(g + 1) * HD]),
            "wv": np.ascontiguousarray(wv[:, g * HD:(g + 1) * HD]),
            "wo": np.ascontiguousarray(wo[g * NH * HD:(g + 1) * NH * HD, :]),
            "cos": cos_t, "sinm": sinm_t, "masks": masks, "ident": ident,
            "onec": np.ones((128, 128), np.float32),
        })
    return in_maps


def kernel(x, wq, wk, wv, wo, attn_mask):
    nc = _build()
    in_maps = _in_maps(x, wq, wk, wv, wo)
    res = run_bass_kernel_spmd(nc, in_maps, list(range(NCORES)))
    out = np.zeros((B, S, E), dtype=np.float64)
    for d in range(NCORES):
        out[d // 4] += res.results[d]["out"]
    return out.astype(np.float32)


# revision 16
# speedup vs baseline: 1.0594x; 1.0594x over previous
"""GroupedQueryAttention TRN2 Bass kernel, 8-way (batch x head-group) parallel.

B=2, S=2048, E=2048, H=16 q-heads, KVH=4 kv-heads, HD=128.
Core d handles batch d//4 and head group g=d%4: q-heads {4g..4g+3} plus
kv-head g.  No replicated projection work: per-core PE load is the perfect
1/8 split of total MACs (vs. head-only sharding which recomputes K/V and
the full-token Q projection on every core).  Each core emits a partial
out[S, E] (summed over its 4 heads' E-slice of out_proj); the host sums
the 4 partials per batch -- device time only pays a 16MB write per core.

Layout strategy (everything transposed so all matmuls use natural layouts):
  phase 1: qT/kT/vT[hd, tok] = W.T @ xT  (lhsT = W chunks, rhs = xT chunks)
           + RoPE applied on PSUM->SBUF epilogue (1/sqrt(HD) folded into wq)
           Tile 0 accumulates all 6 outputs chunk-major (smallest possible
           first-DMA footprint); tiles 1..3 run two 3-output passes
           (gA=q0,q1,k then gB=q2,q3,v) so pass-A drains overlap pass-B
           matmuls and PSUM banks never stall the PE.  v chunks are
           PE-transposed to natural layout right after each tile's drain.
  attention (per head, 512-wide q tile): flash-style over PAIRS of 128-wide
           kt chunks: scoresT[kt, qt] = kT_chunk.T @ qT_tile -> one exp over
           both chunks [128,1024] (no max subtraction; scores are ~N(0,1))
           -> causal mask on diagonal chunks -> ctxT[hd, qt] += v_chunk.T @ P;
           sumexp[1, qt] += ones.T @ P.  PE runs one chunk pair ahead of ACT
           so exp latency is hidden.  Normalize ctxT with broadcast sums
           (all-ones stationary) + reciprocal_approx_fast.
  out_proj (per q tile, right after that tile's attention so its output DMA
           overlaps the next q tile's attention): out[tok, e] = sum_h
           ctxT_h_chunk.T @ wo_h (partial over this core's 512 head dims;
           host sums the 4 partials per batch).

All matmul operands are float32r (full PE rate at N>=512; ~1e-3 precision).
"""
import sys
sys.path.insert(0, '/opt/trn_rl_repo')

import numpy as np
from contextlib import ExitStack

import concourse.bass as bass
import concourse.bacc as bacc
import concourse.tile as tile
from concourse import mybir
from concourse.bass_utils import run_bass_kernel_spmd
from concourse.alu_op_type import AluOpType

F32 = mybir.dt.float32
F32R = mybir.dt.float32r
EXP = mybir.ActivationFunctionType.Exp

B, S, E = 2, 2048, 2048
H, KVH, HD = 16, 4, 128
NH = H // KVH              # 4 q-heads per core (one kv head)
T = B * S
NCORES = 8
NT = 512                   # token tile (matmul free dim)
NTT = S // NT              # 4 token tiles per core
KC = E // 128              # 16 contraction chunks for projections
KB = S // 128              # 16 kt chunks per core batch
ROPE_BASE = 10000.0

_CACHE = {}


def _emit(nc, tc, ctx):
    xT_d = nc.declare_dram_parameter("xT", [E, S], F32R, isOutput=False)
    wq_d = nc.declare_dram_parameter("wq", [E, NH * HD], F32R, isOutput=False)
    wk_d = nc.declare_dram_parameter("wk", [E, HD], F32R, isOutput=False)
    wv_d = nc.declare_dram_parameter("wv", [E, HD], F32R, isOutput=False)
    wo_d = nc.declare_dram_parameter("wo", [NH * HD, E], F32R, isOutput=False)
    cos_d = nc.declare_dram_parameter("cos", [HD, S], F32, isOutput=False)
    sinm_d = nc.declare_dram_parameter("sinm", [HD, S], F32, isOutput=False)
    masks_d = nc.declare_dram_parameter("masks", [4, 128, NT], F32, isOutput=False)
    ident_d = nc.declare_dram_parameter("ident", [128, 128], F32R, isOutput=False)
    onec_d = nc.declare_dram_parameter("onec", [128, 128], F32R, isOutput=False)
    out_d = nc.declare_dram_parameter("out", [S, E], F32, isOutput=True)

    persist = ctx.enter_context(tc.tile_pool(name="persist", bufs=1))
    qTs = persist.tile([HD, NH, S], F32R)       # 4 q heads, transposed
    kT = persist.tile([HD, S], F32R)
    v_sb = persist.tile([128, KB, HD], F32R)    # v natural: [tok%128, blk, hd]
    ident = persist.tile([128, 128], F32R)
    ones_col = persist.tile([128, 128], F32R)
    scr = persist.tile([1, 128], F32)
    nc.sync.dma_start(ident[:], ident_d[:, :])
    nc.sync.dma_start(ones_col[:], onec_d[:, :])
    # preload the exp ACT table set (~2.7us) while the PE is on projections
    nc.scalar.activation(scr[:], ident[0:1, :], EXP)

    # ---------------- phase 1: projections + RoPE ----------------
    with ExitStack() as p1:
        wpool = p1.enter_context(tc.tile_pool(name="wpool", bufs=1))
        trig = p1.enter_context(tc.tile_pool(name="trig", bufs=2))
        xpool = p1.enter_context(tc.tile_pool(name="xpool", bufs=1))
        rope = p1.enter_context(tc.tile_pool(name="rope", bufs=2))
        vTp = p1.enter_context(tc.tile_pool(name="vTp", bufs=2))
        ps1 = p1.enter_context(tc.tile_pool(name="ps1", bufs=1, space="PSUM"))
        pst = p1.enter_context(tc.tile_pool(name="pst", bufs=2, space="PSUM"))

        wq_s = wpool.tile([128, KC, NH * HD], F32R)
        wk_s = wpool.tile([128, KC, HD], F32R)
        wv_s = wpool.tile([128, KC, HD], F32R)
        wqv = wq_d.rearrange("(k p) m -> p k m", p=128)
        wkv = wk_d.rearrange("(k p) m -> p k m", p=128)
        wvv = wv_d.rearrange("(k p) t -> p k t", p=128)
        xT_view = xT_d.rearrange("(k p) t -> p k t", p=128)

        # spread DMAs over the two HW DGE queues (sync + scalar); a single
        # queue saturates around ~250 GB/s, well below HBM bandwidth
        def load_wchunk(kq):
            ks = slice(4 * kq, 4 * kq + 4)
            nc.scalar.dma_start(wq_s[:, ks, :], wqv[:, ks, :])
            nc.sync.dma_start(wk_s[:, ks, :], wkv[:, ks, :])
            nc.sync.dma_start(wv_s[:, ks, :], wvv[:, ks, :])

        def load_xk(tt):
            t0 = tt * NT
            xk = []
            for kq in range(4):  # 4 DMAs x 4 chunks of [128, NT]
                xt = xpool.tile([128, 4, NT], F32R, tag=f"xk{kq}")
                (nc.sync if kq % 2 == 0 else nc.scalar).dma_start(
                    xt[:], xT_view[:, 4 * kq:4 * kq + 4, t0:t0 + NT])
                xk.append(xt)
            return xk

        def load_trig(tt):
            t0 = tt * NT
            cos_c = trig.tile([HD, NT], F32, tag="cos")
            sinm_c = trig.tile([HD, NT], F32, tag="sinm")
            nc.scalar.dma_start(cos_c[:], cos_d[:, t0:t0 + NT])
            nc.scalar.dma_start(sinm_c[:], sinm_d[:, t0:t0 + NT])
            return cos_c, sinm_c

        # issue order = queue order: tile-0 consumes chunk-major, so issue
        # the first quarter's w + x at single-chunk granularity (the PE's
        # first matmul then needs only ~0.6MB and starts right after the
        # ~9us DMA preamble), then w-chunk groups interleaved with tile-0
        # x quarters on both queues so arrivals pace consumption.
        xk0 = []
        for kq in range(4):
            ks = slice(4 * kq, 4 * kq + 4)
            qa, qb = ((nc.sync, nc.scalar) if kq % 2 == 0
                      else (nc.scalar, nc.sync))
            xt = xpool.tile([128, 4, NT], F32R, tag=f"xk{kq}")
            if kq == 0:
                for c in range(4):
                    qa.dma_start(wq_s[:, c, :], wqv[:, c, :])
                    qb.dma_start(xt[:, c, :], xT_view[:, c, 0:NT])
                    qb.dma_start(wk_s[:, c, :], wkv[:, c, :])
                    qa.dma_start(wv_s[:, c, :], wvv[:, c, :])
            else:
                qa.dma_start(wq_s[:, ks, :], wqv[:, ks, :])
                qb.dma_start(xt[:], xT_view[:, ks, 0:NT])
                qb.dma_start(wk_s[:, ks, :], wkv[:, ks, :])
                qa.dma_start(wv_s[:, ks, :], wvv[:, ks, :])
            xk0.append(xt)
        tcur = load_trig(0)
        xk_next = load_xk(1)
        tnext = load_trig(1)

        def rope_drain(psum, dest, cos_c, sinm_c, cpeng):
            # quick PSUM->SBUF release copy (so the bank frees in ~1us and
            # the next tile's matmuls aren't gated on the full RoPE chain),
            # then q' = q*cos + swap_halves(q)*sinm from SBUF at leisure
            cpy = rope.tile([HD, NT], F32, tag="cpy", bufs=3)
            if cpeng == 0:
                nc.vector.tensor_copy(cpy[:], psum[:])
            else:
                nc.scalar.copy(cpy[:], psum[:])
            sw = rope.tile([HD, NT], F32, tag="sw")
            nc.scalar.copy(sw[0:64, :], cpy[64:128, :])
            nc.scalar.copy(sw[64:128, :], cpy[0:64, :])
            nc.vector.tensor_tensor(sw[:], sw[:], sinm_c[:], AluOpType.mult)
            nc.vector.tensor_tensor(dest, cpy[:], cos_c[:], AluOpType.mult)
            nc.vector.tensor_tensor(dest, dest, sw[:], AluOpType.add)

        def v_transpose(vT_t, tt):
            # vT chunk -> natural v in v_sb via PE transpose
            for i in range(NT // 128):
                tp = pst.tile([128, 128], F32R, tag="tp")
                nc.tensor.matmul(tp[:], vT_t[:, i * 128:(i + 1) * 128],
                                 ident[:], is_transpose=True)
                nc.vector.tensor_copy(v_sb[:, tt * (NT // 128) + i, :], tp[:])

        for tt in range(NTT):
            t0 = tt * NT
            xk = xk0 if tt == 0 else xk_next
            cos_c, sinm_c = tcur
            if 0 < tt < NTT - 1:
                xk_next = load_xk(tt + 1)
                tnext = load_trig(tt + 1)
            gA = ps1.tile([HD, 3, NT], F32, tag="gA")
            gB = ps1.tile([HD, 3, NT], F32, tag="gB")
            vT_t = vTp.tile([HD, NT], F32R, tag="vT")

            def wslotsA(k):
                return [(gA, 0, wq_s[:, k, 0:HD]),
                        (gA, 1, wq_s[:, k, HD:2 * HD]),
                        (gA, 2, wk_s[:, k, :])]

            def wslotsB(k):
                return [(gB, 0, wq_s[:, k, 2 * HD:3 * HD]),
                        (gB, 1, wq_s[:, k, 3 * HD:4 * HD]),
                        (gB, 2, wv_s[:, k, :])]

            def mm_pass(slot_fn, drains=()):
                for k in range(KC):
                    xck = xk[k // 4][:, k % 4, :]
                    for (ps, idx, w) in slot_fn(k):
                        nc.tensor.matmul(ps[:, idx, :], w, xck,
                                         start=(k == 0), stop=(k == KC - 1))
                for psum, dest, cpeng in drains:
                    rope_drain(psum, dest, cos_c, sinm_c, cpeng)

            drainsA = [(gA[:, 0, :], qTs[:, 0, t0:t0 + NT], 0),
                       (gA[:, 1, :], qTs[:, 1, t0:t0 + NT], 1),
                       (gA[:, 2, :], kT[:, t0:t0 + NT], 0)]
            if tt == 0:
                # chunk-major: all 6 outputs per chunk -> first matmul only
                # needs w-chunk 0 + the first x quarter (~2.5MB of DMA)
                mm_pass(lambda k: wslotsA(k) + wslotsB(k))
                for psum, dest, cpeng in drainsA:
                    rope_drain(psum, dest, cos_c, sinm_c, cpeng)
            else:
                # two staggered 3-output passes: pass-A drains overlap
                # pass-B matmuls, so PSUM never backpressures the PE
                mm_pass(wslotsA, drainsA)
                mm_pass(wslotsB)
            # vT drains first so the transposes (the next PE work) aren't
            # queued behind the q2/q3 RoPE chains on the scalar engine
            nc.scalar.copy(vT_t[:], gB[:, 2, :])
            v_transpose(vT_t, tt)
            rope_drain(gB[:, 0, :], qTs[:, 2, t0:t0 + NT], cos_c, sinm_c, 0)
            rope_drain(gB[:, 1, :], qTs[:, 3, t0:t0 + NT], cos_c, sinm_c, 0)
            tcur = tnext

    # ---------- attention + out_proj, interleaved per q-tile ----------
    with ExitStack() as p2:
        # phase-2 constants: masks gate the first diagonal exp (~1us in,
        # small), wo gates the first out_proj which the interleaving defers
        # to ~25us in -- both loads ride the now-idle queues
        mpool = p2.enter_context(tc.tile_pool(name="mpool", bufs=1))
        wopool = p2.enter_context(tc.tile_pool(name="wopool", bufs=1))
        masks_s = mpool.tile([128, 4, NT], F32)
        wo_sb = wopool.tile([HD, NH, E], F32R)
        nc.sync.dma_start(masks_s[:], masks_d.rearrange("m p j -> p m j"))
        nc.scalar.dma_start(wo_sb[:], wo_d.rearrange("(h p) e -> p h e", p=128))
        ppool = p2.enter_context(tc.tile_pool(name="ppool", bufs=4))
        npool = p2.enter_context(tc.tile_pool(name="npool", bufs=2))
        cpool = p2.enter_context(tc.tile_pool(name="cpool", bufs=2))
        obpool = p2.enter_context(tc.tile_pool(name="obpool", bufs=2))
        # PSUM budget (8 banks): scores pairs 2x2 | ctx 1 | sumexp 1 |
        # out_proj 2
        pss = p2.enter_context(tc.tile_pool(name="pss", bufs=2, space="PSUM"))
        psc = p2.enter_context(tc.tile_pool(name="psc", bufs=1, space="PSUM"))
        psn = p2.enter_context(tc.tile_pool(name="psn", bufs=1, space="PSUM"))
        pso = p2.enter_context(tc.tile_pool(name="pso", bufs=2, space="PSUM"))

        def emit_outproj_chunk(ctx_p, qt_p, tc4, alt=False):
            # one 128-token chunk of out_proj for a finished q tile; called
            # between attention heads so these always-ready matmuls fill the
            # PE bubbles of the exp-gated attention pipeline.  alt=True (the
            # post-attention tail, when ACT has no more exps) splits the
            # PSUM drains across DVE+ACT so they don't pace the PE.
            tch = qt_p * (NT // 128) + tc4
            tsl = slice(tch * 128, (tch + 1) * 128)
            csl = slice(tc4 * 128, (tc4 + 1) * 128)
            ob = obpool.tile([128, E], F32, tag="ob", bufs=3)
            for ech in range(E // NT):
                esl = slice(ech * NT, (ech + 1) * NT)
                op = pso.tile([128, NT], F32, tag="o")
                for h in range(NH):
                    nc.tensor.matmul(op[:], ctx_p[:, h, csl],
                                     wo_sb[:, h, esl],
                                     start=(h == 0), stop=(h == NH - 1))
                if alt and ech % 2 == 1:
                    nc.scalar.copy(ob[:, esl], op[:])
                else:
                    nc.vector.tensor_copy(ob[:, esl], op[:])
            (nc.sync if tch % 2 == 0 else nc.scalar).dma_start(
                out_d[tsl, :], ob[:])

        pending = []
        for qt in range(NTT):
            npairs = 2 * (qt + 1)
            nk = 2 * npairs
            q_sl = slice(qt * NT, (qt + 1) * NT)
            ctx_t = cpool.tile([HD, NH, NT], F32R, tag="ctx")
            # ---- attention for all 4 heads on this q tile ----
            for h in range(NH):
                if pending:
                    emit_outproj_chunk(*pending.pop(0))
                ctxp = psc.tile([HD, NT], F32, tag="ctx")
                sump = psn.tile([128, NT], F32, tag="sum")

                def delta(kc):
                    # fully-masked column prefix of a diagonal chunk
                    return (kc - 4 * qt) * 128 if kc >= 4 * qt else 0

                def emit_ctx_sum(pexp, padd, j):
                    for half in (0, 1):
                        kc = 2 * j + half
                        dl = delta(kc)
                        st, sp_ = (kc == 0), (kc == nk - 1)
                        nc.tensor.matmul(
                            ctxp[:, dl:], v_sb[:, kc, :],
                            pexp[:, half, dl:], start=st, stop=sp_)
                        if padd is None:  # diagonal pair: per-half sums
                            nc.tensor.matmul(
                                sump[:, dl:], ones_col[:],
                                pexp[:, half, dl:], start=st, stop=sp_)
                    if padd is not None:  # one sum matmul per pair
                        st, sp_ = (j == 0), (j == npairs - 1)
                        nc.tensor.matmul(sump[:], ones_col[:], padd[:],
                                         start=st, stop=sp_)

                prev = None
                for j in range(npairs):
                    diag = j >= 2 * qt
                    sp2 = pss.tile([128, 2, NT], F32, tag="s")
                    for half in (0, 1):
                        kc = 2 * j + half
                        dl = delta(kc)
                        nc.tensor.matmul(
                            sp2[:, half, dl:],
                            kT[:, kc * 128:(kc + 1) * 128],
                            qTs[:, h, q_sl][:, dl:])
                    pexp = ppool.tile([128, 2, NT], F32R, tag="p")
                    padd = None
                    if diag:
                        # per-half exp on valid columns, then causal mask
                        for half in (0, 1):
                            kc = 2 * j + half
                            dl = delta(kc)
                            nc.scalar.activation(pexp[:, half, dl:],
                                                 sp2[:, half, dl:], EXP)
                            nc.vector.tensor_tensor(
                                pexp[:, half, dl:], pexp[:, half, dl:],
                                masks_s[:, kc - 4 * qt, dl:],
                                AluOpType.mult)
                    else:
                        nc.scalar.activation(pexp[:], sp2[:], EXP)
                        padd = ppool.tile([128, NT], F32R, tag="padd")
                        nc.vector.tensor_tensor(
                            padd[:], pexp[:, 0, :], pexp[:, 1, :],
                            AluOpType.add)
                    if prev is not None:
                        emit_ctx_sum(*prev)
                    prev = (pexp, padd, j)
                emit_ctx_sum(*prev)
                # drain PSUM into SBUF immediately so the banks free up;
                # sump already holds sumexp broadcast to all partitions
                # (all-ones stationary), so normalize is just recip+mult
                ctmp = npool.tile([HD, NT], F32, tag="ctmp")
                nc.vector.tensor_copy(ctmp[:], ctxp[:])
                bcs = npool.tile([128, NT], F32, tag="bcs")
                nc.vector.reciprocal_approx_fast(bcs[:], sump[:])
                nc.vector.tensor_tensor(ctx_t[:, h, :], ctmp[:], bcs[:],
                                        AluOpType.mult)

            pending = [(ctx_t, qt, tc4) for tc4 in range(NT // 128)]
        for args in pending:
            emit_outproj_chunk(*args, alt=True)


def _build():
    if "nc" in _CACHE:
        return _CACHE["nc"]
    nc = bacc.Bacc("TRN2", target_bir_lowering=False, debug=False,
                   num_devices=NCORES)
    with tile.TileContext(nc) as tc:
        with nc.allow_low_precision(reason="float32r operands for full-rate PE"):
            with ExitStack() as ctx:
                _emit(nc, tc, ctx)
    nc.compile()
    _CACHE["nc"] = nc
    return nc


def _host_consts():
    if "consts" in _CACHE:
        return _CACHE["consts"]
    # RoPE tables, computed in float32 like the reference
    inv_freq = (1.0 / (ROPE_BASE ** (np.arange(0, HD, 2, dtype=np.float32) / HD))
                ).astype(np.float32)
    t = np.arange(S, dtype=np.float32)
    freqs = np.outer(t, inv_freq).astype(np.float32)          # [S, 64]
    emb = np.concatenate([freqs, freqs], axis=-1)             # [S, HD]
    cos_t = np.ascontiguousarray(np.cos(emb).T.astype(np.float32))  # [HD, S]
    sin_t = np.sin(emb).T.astype(np.float32)
    sinm_t = np.ascontiguousarray(
        np.concatenate([-sin_t[:64], sin_t[64:]], axis=0))
    # causal masks for the 4 diagonal 128-chunk offsets within a 512 q-tile
    p = np.arange(128)[:, None]
    j = np.arange(NT)[None, :]
    masks = np.stack([(m * 128 + p <= j) for m in range(4)]).astype(np.float32)
    ident = np.eye(128, dtype=np.float32)
    _CACHE["consts"] = (cos_t, sinm_t, masks, ident)
    return _CACHE["consts"]


def _in_maps(x, wq, wk, wv, wo):
    cos_t, sinm_t, masks, ident = _host_consts()
    x = np.asarray(x, dtype=np.float32)
    scale = np.float32(1.0 / np.sqrt(HD))
    in_maps = []
    for d in range(NCORES):
        b, g = d // 4, d % 4
        xT = np.ascontiguousarray(x[b].T)                     # [E, S]
        in_maps.append({
            "xT": xT,
            "wq": np.ascontiguousarray(wq[:, g * NH * HD:(g + 1) * NH * HD]) * scale,
            "wk": np.ascontiguousarray(wk[:, g * HD:(g + 1) * HD]),
            "wv": np.ascontiguousarray(wv[:, g * HD:(g + 1) * HD]),
            "wo": np.ascontiguousarray(wo[g * NH * HD:(g + 1) * NH * HD, :]),
            "cos": cos_t, "sinm": sinm_t, "masks": masks, "ident": ident,
            "onec": np.ones((128, 128), np.float32),
        })
    return in_maps


def kernel(x, wq, wk, wv, wo, attn_mask):
    nc = _build()
    in_maps = _in_maps(x, wq, wk, wv, wo)
    res = run_bass_kernel_spmd(nc, in_maps, list(range(NCORES)))
    out = np.zeros((B, S, E), dtype=np.float64)
    for d in range(NCORES):
        out[d // 4] += res.results[d]["out"]
    return out.astype(np.float32)


# revision 25
# speedup vs baseline: 1.0750x; 1.0147x over previous
"""GroupedQueryAttention TRN2 Bass kernel, 8-way (batch x head-group) parallel.

B=2, S=2048, E=2048, H=16 q-heads, KVH=4 kv-heads, HD=128.
Core d handles batch d//4 and head group g=d%4: q-heads {4g..4g+3} plus
kv-head g.  No replicated projection work: per-core PE load is the perfect
1/8 split of total MACs (vs. head-only sharding which recomputes K/V and
the full-token Q projection on every core).  Each core emits a partial
out[S, E] (summed over its 4 heads' E-slice of out_proj); the host sums
the 4 partials per batch -- device time only pays a 16MB write per core.

Layout strategy (everything transposed so all matmuls use natural layouts):
  phase 1: qT/kT/vT[hd, tok] = W.T @ xT  (lhsT = W chunks, rhs = xT chunks)
           + RoPE applied on PSUM->SBUF epilogue (1/sqrt(HD) folded into wq)
           Tile 0 accumulates all 6 outputs chunk-major (smallest possible
           first-DMA footprint); tiles 1..3 run two 3-output passes
           (gA=q0,q1,k then gB=q2,q3,v) so pass-A drains overlap pass-B
           matmuls and PSUM banks never stall the PE.  v chunks are
           PE-transposed to natural layout right after each tile's drain.
  attention (per head, 512-wide q tile): flash-style over PAIRS of 128-wide
           kt chunks: scoresT[kt, qt] = kT_chunk.T @ qT_tile -> one exp over
           both chunks [128,1024] (no max subtraction; scores are ~N(0,1))
           -> causal mask on diagonal chunks -> ctxT[hd, qt] += v_chunk.T @ P;
           sumexp[1, qt] += ones.T @ P.  PE runs one chunk pair ahead of ACT
           so exp latency is hidden.  Normalize ctxT with broadcast sums
           (all-ones stationary) + reciprocal_approx_fast.
  out_proj (per q tile, right after that tile's attention so its output DMA
           overlaps the next q tile's attention): out[tok, e] = sum_h
           ctxT_h_chunk.T @ wo_h (partial over this core's 512 head dims;
           host sums the 4 partials per batch).

All matmul operands are float32r (full PE rate at N>=512; ~1e-3 precision).
"""
import sys
sys.path.insert(0, '/opt/trn_rl_repo')

import numpy as np
from contextlib import ExitStack

import concourse.bass as bass
import concourse.bacc as bacc
import concourse.tile as tile
from concourse import mybir
from concourse.bass_utils import run_bass_kernel_spmd
from concourse.alu_op_type import AluOpType

F32 = mybir.dt.float32
F32R = mybir.dt.float32r
EXP = mybir.ActivationFunctionType.Exp

B, S, E = 2, 2048, 2048
H, KVH, HD = 16, 4, 128
NH = H // KVH              # 4 q-heads per core (one kv head)
T = B * S
NCORES = 8
NT = 512                   # token tile (matmul free dim)
NTT = S // NT              # 4 token tiles per core
KC = E // 128              # 16 contraction chunks for projections
KB = S // 128              # 16 kt chunks per core batch
ROPE_BASE = 10000.0

_CACHE = {}


def _emit(nc, tc, ctx):
    xT_d = nc.declare_dram_parameter("xT", [E, S], F32R, isOutput=False)
    wq_d = nc.declare_dram_parameter("wq", [E, NH * HD], F32R, isOutput=False)
    wk_d = nc.declare_dram_parameter("wk", [E, HD], F32R, isOutput=False)
    wv_d = nc.declare_dram_parameter("wv", [E, HD], F32R, isOutput=False)
    wo_d = nc.declare_dram_parameter("wo", [NH * HD, E], F32R, isOutput=False)
    cos_d = nc.declare_dram_parameter("cos", [HD, S], F32, isOutput=False)
    sinm_d = nc.declare_dram_parameter("sinm", [HD, S], F32, isOutput=False)
    tri_d = nc.declare_dram_parameter("tri", [128, 128], F32, isOutput=False)
    ident_d = nc.declare_dram_parameter("ident", [128, 128], F32R, isOutput=False)
    onec_d = nc.declare_dram_parameter("onec", [128, 128], F32R, isOutput=False)
    out_d = nc.declare_dram_parameter("out", [S, E], F32, isOutput=True)

    persist = ctx.enter_context(tc.tile_pool(name="persist", bufs=1))
    qTs = persist.tile([HD, NH, S], F32R)       # 4 q heads, transposed
    kT = persist.tile([HD, S], F32R)
    v_sb = persist.tile([128, KB, HD], F32R)    # v natural: [tok%128, blk, hd]
    ident = persist.tile([128, 128], F32R)
    ones_col = persist.tile([128, 128], F32R)
    tri_m = persist.tile([128, 128], F32)
    scr = persist.tile([1, 128], F32)
    nc.sync.dma_start(ident[:], ident_d[:, :])
    nc.sync.dma_start(ones_col[:], onec_d[:, :])
    nc.sync.dma_start(tri_m[:], tri_d[:, :])
    # preload the exp ACT table set (~2.7us) while the PE is on projections
    nc.scalar.activation(scr[:], ident[0:1, :], EXP)

    # ---------------- phase 1: projections + RoPE ----------------
    with ExitStack() as p1:
        wpool = p1.enter_context(tc.tile_pool(name="wpool", bufs=1))
        trig = p1.enter_context(tc.tile_pool(name="trig", bufs=2))
        xpool = p1.enter_context(tc.tile_pool(name="xpool", bufs=1))
        rope = p1.enter_context(tc.tile_pool(name="rope", bufs=2))
        vTp = p1.enter_context(tc.tile_pool(name="vTp", bufs=2))
        ps1 = p1.enter_context(tc.tile_pool(name="ps1", bufs=1, space="PSUM"))
        pst = p1.enter_context(tc.tile_pool(name="pst", bufs=2, space="PSUM"))

        wq_s = wpool.tile([128, KC, NH * HD], F32R)
        wk_s = wpool.tile([128, KC, HD], F32R)
        wv_s = wpool.tile([128, KC, HD], F32R)
        wqv = wq_d.rearrange("(k p) m -> p k m", p=128)
        wkv = wk_d.rearrange("(k p) m -> p k m", p=128)
        wvv = wv_d.rearrange("(k p) t -> p k t", p=128)
        xT_view = xT_d.rearrange("(k p) t -> p k t", p=128)

        # spread DMAs over the two HW DGE queues (sync + scalar); a single
        # queue saturates around ~250 GB/s, well below HBM bandwidth
        def load_wchunk(kq):
            ks = slice(4 * kq, 4 * kq + 4)
            nc.scalar.dma_start(wq_s[:, ks, :], wqv[:, ks, :])
            nc.sync.dma_start(wk_s[:, ks, :], wkv[:, ks, :])
            nc.sync.dma_start(wv_s[:, ks, :], wvv[:, ks, :])

        def load_xk(tt):
            t0 = tt * NT
            xk = []
            for kq in range(4):  # 4 DMAs x 4 chunks of [128, NT]
                xt = xpool.tile([128, 4, NT], F32R, tag=f"xk{kq}")
                (nc.sync if kq % 2 == 0 else nc.scalar).dma_start(
                    xt[:], xT_view[:, 4 * kq:4 * kq + 4, t0:t0 + NT])
                xk.append(xt)
            return xk

        def load_trig(tt):
            t0 = tt * NT
            cos_c = trig.tile([HD, NT], F32, tag="cos")
            sinm_c = trig.tile([HD, NT], F32, tag="sinm")
            nc.scalar.dma_start(cos_c[:], cos_d[:, t0:t0 + NT])
            nc.scalar.dma_start(sinm_c[:], sinm_d[:, t0:t0 + NT])
            return cos_c, sinm_c

        # issue order = queue order: tile-0 consumes chunk-major, so issue
        # the first quarter's w + x at single-chunk granularity (the PE's
        # first matmul then needs only ~0.6MB and starts right after the
        # ~9us DMA preamble), then w-chunk groups interleaved with tile-0
        # x quarters on both queues so arrivals pace consumption.
        xk0 = []
        for kq in range(4):
            ks = slice(4 * kq, 4 * kq + 4)
            qa, qb = ((nc.sync, nc.scalar) if kq % 2 == 0
                      else (nc.scalar, nc.sync))
            xt = xpool.tile([128, 4, NT], F32R, tag=f"xk{kq}")
            if kq == 0:
                for c in range(4):
                    qa.dma_start(wq_s[:, c, :], wqv[:, c, :])
                    qb.dma_start(xt[:, c, :], xT_view[:, c, 0:NT])
                    qb.dma_start(wk_s[:, c, :], wkv[:, c, :])
                    qa.dma_start(wv_s[:, c, :], wvv[:, c, :])
            else:
                qa.dma_start(wq_s[:, ks, :], wqv[:, ks, :])
                qb.dma_start(xt[:], xT_view[:, ks, 0:NT])
                qb.dma_start(wk_s[:, ks, :], wkv[:, ks, :])
                qa.dma_start(wv_s[:, ks, :], wvv[:, ks, :])
            xk0.append(xt)
        tcur = load_trig(0)
        xk_next = load_xk(1)
        tnext = load_trig(1)

        def rope_drain(psum, dest, cos_c, sinm_c, cpeng):
            # quick PSUM->SBUF release copy (so the bank frees in ~1us and
            # the next tile's matmuls aren't gated on the full RoPE chain),
            # then q' = q*cos + swap_halves(q)*sinm from SBUF at leisure
            cpy = rope.tile([HD, NT], F32, tag="cpy", bufs=3)
            if cpeng == 0:
                nc.vector.tensor_copy(cpy[:], psum[:])
            else:
                nc.scalar.copy(cpy[:], psum[:])
            sw = rope.tile([HD, NT], F32, tag="sw")
            nc.scalar.copy(sw[0:64, :], cpy[64:128, :])
            nc.scalar.copy(sw[64:128, :], cpy[0:64, :])
            nc.vector.tensor_tensor(sw[:], sw[:], sinm_c[:], AluOpType.mult)
            nc.vector.tensor_tensor(dest, cpy[:], cos_c[:], AluOpType.mult)
            nc.vector.tensor_tensor(dest, dest, sw[:], AluOpType.add)

        def v_transpose(vT_t, tt):
            # vT chunk -> natural v in v_sb via PE transpose
            for i in range(NT // 128):
                tp = pst.tile([128, 128], F32R, tag="tp")
                nc.tensor.matmul(tp[:], vT_t[:, i * 128:(i + 1) * 128],
                                 ident[:], is_transpose=True)
                nc.vector.tensor_copy(v_sb[:, tt * (NT // 128) + i, :], tp[:])

        for tt in range(NTT):
            t0 = tt * NT
            xk = xk0 if tt == 0 else xk_next
            cos_c, sinm_c = tcur
            if 0 < tt < NTT - 1:
                xk_next = load_xk(tt + 1)
                tnext = load_trig(tt + 1)
            gA = ps1.tile([HD, 3, NT], F32, tag="gA")
            gB = ps1.tile([HD, 3, NT], F32, tag="gB")
            vT_t = vTp.tile([HD, NT], F32R, tag="vT")

            def wslotsA(k):
                return [(gA, 0, wq_s[:, k, 0:HD]),
                        (gA, 1, wq_s[:, k, HD:2 * HD]),
                        (gA, 2, wk_s[:, k, :])]

            def wslotsB(k):
                return [(gB, 0, wq_s[:, k, 2 * HD:3 * HD]),
                        (gB, 1, wq_s[:, k, 3 * HD:4 * HD]),
                        (gB, 2, wv_s[:, k, :])]

            def mm_pass(slot_fn, drains=()):
                for k in range(KC):
                    xck = xk[k // 4][:, k % 4, :]
                    for (ps, idx, w) in slot_fn(k):
                        nc.tensor.matmul(ps[:, idx, :], w, xck,
                                         start=(k == 0), stop=(k == KC - 1))
                for psum, dest, cpeng in drains:
                    rope_drain(psum, dest, cos_c, sinm_c, cpeng)

            drainsA = [(gA[:, 0, :], qTs[:, 0, t0:t0 + NT], 0),
                       (gA[:, 1, :], qTs[:, 1, t0:t0 + NT], 1),
                       (gA[:, 2, :], kT[:, t0:t0 + NT], 0)]
            if tt == 0:
                # chunk-major: all 6 outputs per chunk -> first matmul only
                # needs w-chunk 0 + the first x quarter (~2.5MB of DMA)
                mm_pass(lambda k: wslotsA(k) + wslotsB(k))
                for psum, dest, cpeng in drainsA:
                    rope_drain(psum, dest, cos_c, sinm_c, cpeng)
            else:
                # two staggered 3-output passes: pass-A drains overlap
                # pass-B matmuls, so PSUM never backpressures the PE
                mm_pass(wslotsA, drainsA)
                mm_pass(wslotsB)
            # vT drains first so the transposes (the next PE work) aren't
            # queued behind the q2/q3 RoPE chains on the scalar engine
            nc.scalar.copy(vT_t[:], gB[:, 2, :])
            v_transpose(vT_t, tt)
            rope_drain(gB[:, 0, :], qTs[:, 2, t0:t0 + NT], cos_c, sinm_c, 0)
            rope_drain(gB[:, 1, :], qTs[:, 3, t0:t0 + NT], cos_c, sinm_c, 0)
            tcur = tnext

    # ---------- attention + out_proj, interleaved per q-tile ----------
    with ExitStack() as p2:
        # wo gates the first out_proj, which the interleaving defers to
        # ~25us into phase 2 -- plenty of slack on the now-idle queues
        wopool = p2.enter_context(tc.tile_pool(name="wopool", bufs=1))
        wo_sb = wopool.tile([HD, NH, E], F32R)
        nc.scalar.dma_start(wo_sb[:], wo_d.rearrange("(h p) e -> p h e", p=128))
        ppool = p2.enter_context(tc.tile_pool(name="ppool", bufs=4))
        npool = p2.enter_context(tc.tile_pool(name="npool", bufs=2))
        cpool = p2.enter_context(tc.tile_pool(name="cpool", bufs=2))
        obpool = p2.enter_context(tc.tile_pool(name="obpool", bufs=2))
        # PSUM budget (8 banks): scores pairs 2x2 | ctx 1 | sumexp 1 |
        # out_proj 2
        pss = p2.enter_context(tc.tile_pool(name="pss", bufs=2, space="PSUM"))
        psc = p2.enter_context(tc.tile_pool(name="psc", bufs=1, space="PSUM"))
        psn = p2.enter_context(tc.tile_pool(name="psn", bufs=1, space="PSUM"))
        pso = p2.enter_context(tc.tile_pool(name="pso", bufs=2, space="PSUM"))

        def emit_outproj_chunk(ctx_p, qt_p, tc4, alt=False):
            # one 128-token chunk of out_proj for a finished q tile; called
            # between attention heads so these always-ready matmuls fill the
            # PE bubbles of the exp-gated attention pipeline.  alt=True (the
            # post-attention tail, when ACT has no more exps) splits the
            # PSUM drains across DVE+ACT so they don't pace the PE.
            tch = qt_p * (NT // 128) + tc4
            tsl = slice(tch * 128, (tch + 1) * 128)
            csl = slice(tc4 * 128, (tc4 + 1) * 128)
            ob = obpool.tile([128, E], F32, tag="ob", bufs=3)
            for ech in range(E // NT):
                esl = slice(ech * NT, (ech + 1) * NT)
                op = pso.tile([128, NT], F32, tag="o")
                for h in range(NH):
                    nc.tensor.matmul(op[:], ctx_p[:, h, csl],
                                     wo_sb[:, h, esl],
                                     start=(h == 0), stop=(h == NH - 1))
                if alt and ech % 2 == 1:
                    nc.scalar.copy(ob[:, esl], op[:])
                else:
                    nc.vector.tensor_copy(ob[:, esl], op[:])
                if alt:
                    # tail: nothing left to overlap the write behind, so
                    # stream each e-slice out as soon as it's drained
                    (nc.sync if ech % 2 == 0 else nc.scalar).dma_start(
                        out_d[tsl, esl], ob[:, esl])
            if not alt:
                (nc.sync if tch % 2 == 0 else nc.scalar).dma_start(
                    out_d[tsl, :], ob[:])

        pending = []
        for qt in range(NTT):
            npairs = 2 * (qt + 1)
            nk = 2 * npairs
            q_sl = slice(qt * NT, (qt + 1) * NT)
            ctx_t = cpool.tile([HD, NH, NT], F32R, tag="ctx")
            # ---- attention for all 4 heads on this q tile ----
            for h in range(NH):
                if pending:
                    emit_outproj_chunk(*pending.pop(0))
                ctxp = psc.tile([HD, NT], F32, tag="ctx")
                sump = psn.tile([128, NT], F32, tag="sum")

                def delta(kc):
                    # fully-masked column prefix of a diagonal chunk
                    return (kc - 4 * qt) * 128 if kc >= 4 * qt else 0

                def emit_ctx_sum(pexp, padd, j):
                    for half in (0, 1):
                        kc = 2 * j + half
                        dl = delta(kc)
                        st, sp_ = (kc == 0), (kc == nk - 1)
                        nc.tensor.matmul(
                            ctxp[:, dl:], v_sb[:, kc, :],
                            pexp[:, half, dl:], start=st, stop=sp_)
                        if padd is None:  # diagonal pair: per-half sums
                            nc.tensor.matmul(
                                sump[:, dl:], ones_col[:],
                                pexp[:, half, dl:], start=st, stop=sp_)
                    if padd is not None:  # one sum matmul per pair
                        st, sp_ = (j == 0), (j == npairs - 1)
                        nc.tensor.matmul(sump[:], ones_col[:], padd[:],
                                         start=st, stop=sp_)

                prev = None
                for j in range(npairs):
                    diag = j >= 2 * qt
                    sp2 = pss.tile([128, 2, NT], F32, tag="s")
                    for half in (0, 1):
                        kc = 2 * j + half
                        dl = delta(kc)
                        nc.tensor.matmul(
                            sp2[:, half, dl:],
                            kT[:, kc * 128:(kc + 1) * 128],
                            qTs[:, h, q_sl][:, dl:])
                    pexp = ppool.tile([128, 2, NT], F32R, tag="p")
                    padd = None
                    if diag:
                        # per-half exp on valid columns; the causal mask
                        # only bites in the first 128 columns (the block on
                        # the diagonal -- one lower-triangle pattern shared
                        # by every chunk offset), so mask just that block
                        for half in (0, 1):
                            kc = 2 * j + half
                            dl = delta(kc)
                            nc.scalar.activation(pexp[:, half, dl:],
                                                 sp2[:, half, dl:], EXP)
                            nc.vector.tensor_tensor(
                                pexp[:, half, dl:dl + 128],
                                pexp[:, half, dl:dl + 128],
                                tri_m[:], AluOpType.mult)
                    else:
                        nc.scalar.activation(pexp[:], sp2[:], EXP)
                        padd = ppool.tile([128, NT], F32R, tag="padd")
                        nc.vector.tensor_tensor(
                            padd[:], pexp[:, 0, :], pexp[:, 1, :],
                            AluOpType.add)
                    if prev is not None:
                        emit_ctx_sum(*prev)
                    prev = (pexp, padd, j)
                emit_ctx_sum(*prev)
                # sump holds sumexp broadcast to all partitions (all-ones
                # stationary), so normalize is recip + one mult straight
                # from PSUM; the next head's first ctx matmul lands ~2us
                # after the head switch, covering the longer bank hold
                bcs = npool.tile([128, NT], F32, tag="bcs")
                nc.vector.reciprocal_approx_fast(bcs[:], sump[:])
                nc.vector.tensor_tensor(ctx_t[:, h, :], ctxp[:], bcs[:],
                                        AluOpType.mult)

            pending = [(ctx_t, qt, tc4) for tc4 in range(NT // 128)]
        for args in pending:
            emit_outproj_chunk(*args, alt=True)


def _build():
    if "nc" in _CACHE:
        return _CACHE["nc"]
    nc = bacc.Bacc("TRN2", target_bir_lowering=False, debug=False,
                   num_devices=NCORES)
    with tile.TileContext(nc) as tc:
        with nc.allow_low_precision(reason="float32r operands for full-rate PE"):
            with ExitStack() as ctx:
                _emit(nc, tc, ctx)
    nc.compile()
    _CACHE["nc"] = nc
    return nc


def _host_consts():
    if "consts" in _CACHE:
        return _CACHE["consts"]
    # RoPE tables, computed in float32 like the reference
    inv_freq = (1.0 / (ROPE_BASE ** (np.arange(0, HD, 2, dtype=np.float32) / HD))
                ).astype(np.float32)
    t = np.arange(S, dtype=np.float32)
    freqs = np.outer(t, inv_freq).astype(np.float32)          # [S, 64]
    emb = np.concatenate([freqs, freqs], axis=-1)             # [S, HD]
    cos_t = np.ascontiguousarray(np.cos(emb).T.astype(np.float32))  # [HD, S]
    sin_t = np.sin(emb).T.astype(np.float32)
    sinm_t = np.ascontiguousarray(
        np.concatenate([-sin_t[:64], sin_t[64:]], axis=0))
    # lower-triangle causal mask for the 128-col block on the diagonal
    p = np.arange(128)[:, None]
    j = np.arange(128)[None, :]
    tri = (p <= j).astype(np.float32)
    ident = np.eye(128, dtype=np.float32)
    _CACHE["consts"] = (cos_t, sinm_t, tri, ident)
    return _CACHE["consts"]


def _in_maps(x, wq, wk, wv, wo):
    cos_t, sinm_t, tri, ident = _host_consts()
    x = np.asarray(x, dtype=np.float32)
    scale = np.float32(1.0 / np.sqrt(HD))
    in_maps = []
    for d in range(NCORES):
        b, g = d // 4, d % 4
        xT = np.ascontiguousarray(x[b].T)                     # [E, S]
        in_maps.append({
            "xT": xT,
            "wq": np.ascontiguousarray(wq[:, g * NH * HD:(g + 1) * NH * HD]) * scale,
            "wk": np.ascontiguousarray(wk[:, g * HD:(g + 1) * HD]),
            "wv": np.ascontiguousarray(wv[:, g * HD:(g + 1) * HD]),
            "wo": np.ascontiguousarray(wo[g * NH * HD:(g + 1) * NH * HD, :]),
            "cos": cos_t, "sinm": sinm_t, "tri": tri, "ident": ident,
            "onec": np.ones((128, 128), np.float32),
        })
    return in_maps


def kernel(x, wq, wk, wv, wo, attn_mask):
    nc = _build()
    in_maps = _in_maps(x, wq, wk, wv, wo)
    res = run_bass_kernel_spmd(nc, in_maps, list(range(NCORES)))
    out = np.zeros((B, S, E), dtype=np.float64)
    for d in range(NCORES):
        out[d // 4] += res.results[d]["out"]
    return out.astype(np.float32)
